# revision 1
# baseline (speedup 1.0000x reference)
"""Trainium2 Bass kernel for nn_CGPODE (graph ODE message passing).

Math: reference computes NFE=8 Euler steps of dx/dt = A x over the node
dim (s_t = M^t x with M = I + 0.125*adj applied on the V axis), concats
the 9 states channel-wise, then applies a 1x1 conv (channel GEMM W) + b.

Refactoring used here (per batch n and lag l, x_nl is a [C,V] slab):
    out_nl = sum_t  W_t s_t           (W_t = W[:, t*C:(t+1)*C])
           = sum_t  M^t (W_t x_nl)    (channel mix commutes with node mix)
           = Horner: u_8 = z_8; u_t = z_t + M u_{t+1}; out_nl = u_0
with z_t = W_t-channel-mix of x computed straight from x via the PE
(x slab as the stationary operand), so every tensor stays node-on-partition
and no transposes or state materialization are needed.

Schedule: per batch n, two half-blocks of LH=6 lags. z for the next
half-block (PE matmuls + ACT psum->sbuf copies) overlaps the current
half-block's Horner sweep (PE matmuls + DVE adds); z is double-buffered.

Sharding: data-parallel over batch N across the 8 cores (adj/W replicated).
All matmuls run as float32r (FP22 multiply, FP32 accumulate) at full PE rate.
"""
import sys
if "/opt/trn_rl_repo" not in sys.path:
    sys.path.append("/opt/trn_rl_repo")  # fallback when axon_site paths absent
from contextlib import ExitStack

import numpy as np

import concourse.bacc as bacc
import concourse.tile as tile
from concourse import mybir
from concourse.bass_utils import run_bass_kernel_spmd

F32 = mybir.dt.float32
F32R = mybir.dt.float32r
F16 = mybir.dt.float16
COPY = mybir.ActivationFunctionType.Copy

NFE = 8
STEP = 0.125
N, C, V, L = 64, 64, 500, 12
O = 64
T = NFE + 1          # 9 states
NCORES = 8
NPC = N // NCORES    # 8 batches per core
WT = 4               # node-dim tiles
VTILE = V // WT      # 125
LH = 6               # lags per half-block (cols per Horner matmul = LH*O = 384)
NHB = L // LH        # 2 half-blocks per batch
JT = T * O           # 576 z columns (t*O + o)
JH = JT // 2         # 288, half of the z columns per psum bank
import os
PACK_Z = os.environ.get("PACK_Z", "1") == "1"  # z matmuls as two concurrent K=64 PE row-groups
USE_F16 = os.environ.get("DTYPE", "f16") == "f16"  # fp16 operands: LDWEIGHTS overlaps (fp32r self-load serializes)
DT = F16 if USE_F16 else F32R
NPDT = np.float16 if USE_F16 else np.float32


def build_nc(repeat=1):
    nc = bacc.Bacc(trn_type="TRN2", target_bir_lowering=False, debug=False)
    x_d = nc.dram_tensor("x", [NPC, C, V, L], DT, kind="ExternalInput")
    mt_d = nc.dram_tensor("mt", [V, V], DT, kind="ExternalInput")
    wr_d = nc.dram_tensor("wr", [C, JT], DT, kind="ExternalInput")
    out_d = nc.dram_tensor("out", [NPC, V, L, O], DT, kind="ExternalOutput")

    with tile.TileContext(nc) as tc, ExitStack() as ctx:
        rep = ctx.enter_context(tc.For_i(0, repeat, 1)) if repeat > 1 else None
        const = ctx.enter_context(tc.tile_pool(name="const", bufs=1))
        xp = ctx.enter_context(tc.tile_pool(name="xp", bufs=2))
        zp = ctx.enter_context(tc.tile_pool(name="zp", bufs=2))
        up = ctx.enter_context(tc.tile_pool(name="up", bufs=12))
        zps = ctx.enter_context(tc.tile_pool(name="zps", bufs=2, space="PSUM"))
        hp = ctx.enter_context(tc.tile_pool(name="hp", bufs=4, space="PSUM"))

        # constants: M^T node-mix tiles and the permuted W
        mt_sb = []
        for wt in range(WT):
            t_ = const.tile([VTILE, V], DT, tag=f"mt{wt}", name=f"mt_sb{wt}")
            nc.sync.dma_start(t_[:], mt_d.ap()[wt * VTILE:(wt + 1) * VTILE, :])
            mt_sb.append(t_)
        if PACK_Z:
            wr_sb = const.tile([2 * C, JT], DT, tag="wr", name="wr_sb")
            nc.sync.dma_start(wr_sb[0:C, :], wr_d.ap()[:])
            nc.sync.dma_start(wr_sb[C:2 * C, :], wr_d.ap()[:])
        else:
            wr_sb = const.tile([C, JT], DT, tag="wr", name="wr_sb")
            nc.sync.dma_start(wr_sb[:], wr_d.ap()[:])

        hbs = [(n, hb) for n in range(NPC) for hb in range(NHB)]
        x_tiles = {}
        z_tiles = {}

        def ensure_x(n):
            # With PACK_Z, x is duplicated onto partitions 64..127 so two
            # lags can occupy distinct PE row-groups in concurrent matmuls.
            if n not in x_tiles:
                if PACK_Z:
                    x_tiles[n] = xp.tile([2 * C, V, L], DT, tag="x",
                                         name=f"x_sb_{n}")
                    nc.sync.dma_start(x_tiles[n][0:C], x_d.ap()[n])
                    nc.sync.dma_start(x_tiles[n][C:2 * C], x_d.ap()[n])
                else:
                    x_tiles[n] = xp.tile([C, V, L], DT, tag="x",
                                         name=f"x_sb_{n}")
                    nc.sync.dma_start(x_tiles[n][:], x_d.ap()[n])
            return x_tiles[n]

        def _zcopy(z, wt, li, ps):
            src = ps[:].rearrange("p (b d) -> p b d", b=2)[:, :, 0:JH]
            dst = z[wt][:, li, :].rearrange("p (b d) -> p b d", b=2)
            nc.scalar.activation(dst, src, COPY)

        def make_z_units(j):
            """Closures that emit half-block j's z work:
            z[wt][:, li, t*O+o] = sum_c x[c, w, lo+li] * W[o, t*C+c]."""
            n, hb = hbs[j]
            x_sb = ensure_x(n)
            z = [zp.tile([VTILE, LH, JT], DT, tag=f"z{wt}",
                         name=f"z{wt}_{n}_{hb}") for wt in range(WT)]
            z_tiles[j] = z
            units = []
            if PACK_Z:
                # lag pair (lp, lp+LH//2) on PE row-groups (0,0) / (64,0)
                for lp in range(LH // 2):
                    la, lb = hb * LH + lp, hb * LH + lp + LH // 2
                    for wt in range(WT):
                        def unit(lp=lp, la=la, lb=lb, wt=wt):
                            ws = slice(wt * VTILE, (wt + 1) * VTILE)
                            psa = zps.tile([VTILE, 1024], F32, tag="zps",
                                           name=f"zpa_{n}_{hb}_{lp}_{wt}")
                            psb = zps.tile([VTILE, 1024], F32, tag="zps",
                                           name=f"zpb_{n}_{hb}_{lp}_{wt}")
                            for h in range(2):
                                cs = slice(h * JH, (h + 1) * JH)
                                po = h * 512
                                nc.tensor.matmul(
                                    psa[:, po:po + JH], x_sb[0:C, ws, la],
                                    wr_sb[0:C, cs], start=True, stop=True,
                                    tile_position=(0, 0))
                                nc.tensor.matmul(
                                    psb[:, po:po + JH], x_sb[C:2 * C, ws, lb],
                                    wr_sb[C:2 * C, cs], start=True, stop=True,
                                    tile_position=(64, 0))
                            _zcopy(z, wt, lp, psa)
                            _zcopy(z, wt, lp + LH // 2, psb)
                        units.append(unit)
            else:
                for li in range(LH):
                    l = hb * LH + li
                    for wt in range(WT):
                        def unit(li=li, l=l, wt=wt):
                            lhsT = x_sb[:, wt * VTILE:(wt + 1) * VTILE, l]
                            ps = zps.tile([VTILE, 1024], F32, tag="zps",
                                          name=f"zps_{n}_{hb}_{li}_{wt}")
                            nc.tensor.matmul(ps[:, 0:JH], lhsT, wr_sb[:, 0:JH],
                                             start=True, stop=True)
                            nc.tensor.matmul(ps[:, 512:512 + JH], lhsT,
                                             wr_sb[:, JH:JT],
                                             start=True, stop=True)
                            _zcopy(z, wt, li, ps)
                        units.append(unit)
            return units

        # prologue: z for half-block 0 emitted standalone
        for unit in make_z_units(0):
            unit()

        for k, (n, hb) in enumerate(hbs):
            pending = make_z_units(k + 1) if k + 1 < len(hbs) else []
            pace = -(-len(pending) // NFE) if pending else 0  # units per step
            z = z_tiles[k]
            # Horner: u_8 = z_8 (read in place); u_t = z_t + M u_{t+1}
            u = [None] * WT
            for t in range(NFE - 1, -1, -1):
                u_new = [None] * WT
                for vt in range(WT):
                    lhs_col = slice(vt * VTILE, (vt + 1) * VTILE)
                    ps = hp.tile([VTILE, LH, O], F32, tag="hp",
                                 name=f"hps_{n}_{hb}_{t}_{vt}")
                    for wt in range(WT):
                        rhs = (z[wt][:, :, NFE * O:T * O]
                               if t == NFE - 1 else u[wt][:])
                        nc.tensor.matmul(ps[:], mt_sb[wt][:, lhs_col], rhs,
                                         start=(wt == 0), stop=(wt == WT - 1))
                    u_new[vt] = up.tile([VTILE, LH, O], DT, tag="u",
                                        name=f"u_{n}_{hb}_{t}_{vt}")
                    nc.vector.tensor_add(
                        u_new[vt][:], ps[:],
                        z[vt][:, :, t * O:(t + 1) * O])
                u = u_new
                # next half-block's z production fills PE step gaps
                for _ in range(pace):
                    if pending:
                        pending.pop(0)()
            while pending:
                pending.pop(0)()
            del z_tiles[k]

            for vt in range(WT):
                vs = slice(vt * VTILE, (vt + 1) * VTILE)
                nc.sync.dma_start(out_d.ap()[n, vs, hb * LH:(hb + 1) * LH, :],
                                  u[vt][:])
    nc.compile()
    return nc


_NC_CACHE = None


def _get_nc(repeat=1):
    global _NC_CACHE
    if _NC_CACHE is None or _NC_CACHE[0] != repeat:
        _NC_CACHE = (repeat, build_nc(repeat))
    return _NC_CACHE[1]


def kernel(x, adj, W, b, _trace=False, _trace_kwargs=None, _repeat=1):
    x = np.ascontiguousarray(np.asarray(x, dtype=np.float32))
    adj = np.asarray(adj, dtype=np.float32)
    W = np.asarray(W, dtype=np.float32)
    b = np.asarray(b, dtype=np.float32)

    mt = np.ascontiguousarray((np.eye(V, dtype=np.float32) + STEP * adj).T.astype(NPDT))
    wr = np.ascontiguousarray(
        W.reshape(O, T, C).transpose(2, 1, 0).reshape(C, JT).astype(NPDT))
    x = x.astype(NPDT)

    nc = _get_nc(_repeat)
    in_maps = [
        {"x": x[i * NPC:(i + 1) * NPC], "mt": mt, "wr": wr}
        for i in range(NCORES)
    ]
    kw = {}
    if _trace:
        kw["trace"] = True
        kw.update(_trace_kwargs or {})
    res = run_bass_kernel_spmd(nc, in_maps, list(range(NCORES)), **kw)
    out = np.concatenate([res.results[i]["out"] for i in range(NCORES)], axis=0)
    out = out.astype(np.float32).transpose(0, 3, 1, 2)   # [N, O, V, L]
    out = out + b[None, :, None, None]
    if _trace:
        return np.ascontiguousarray(out.astype(np.float32)), res
    return np.ascontiguousarray(out.astype(np.float32))



# revision 5
# speedup vs baseline: 1.0419x; 1.0419x over previous
"""Trainium2 Bass kernel for nn_CGPODE (graph ODE message passing).

Math: reference computes NFE=8 Euler steps of dx/dt = A x over the node
dim (s_t = M^t x with M = I + 0.125*adj applied on the V axis), concats
the 9 states channel-wise, then applies a 1x1 conv (channel GEMM W) + b.

Refactoring used here (per batch n and lag l, x_nl is a [C,V] slab):
    out_nl = sum_t  W_t s_t           (W_t = W[:, t*C:(t+1)*C])
           = sum_t  M^t (W_t x_nl)    (channel mix commutes with node mix)
           = Horner: u_8 = z_8; u_t = z_t + M u_{t+1}; out_nl = u_0
with z_t = W_t-channel-mix of x computed straight from x via the PE
(x slab as the stationary operand), so every tensor stays node-on-partition
and no transposes or state materialization are needed.

Schedule: per batch n, two half-blocks of LH=6 lags. z for the next
half-block (PE matmuls + ACT psum->sbuf copies) overlaps the current
half-block's Horner sweep (PE matmuls + DVE adds); z is double-buffered.

Sharding: data-parallel over batch N across the 8 cores (adj/W replicated).
All matmuls run as float32r (FP22 multiply, FP32 accumulate) at full PE rate.
"""
import sys
if "/opt/trn_rl_repo" not in sys.path:
    sys.path.append("/opt/trn_rl_repo")  # fallback when axon_site paths absent
from contextlib import ExitStack

import numpy as np

import concourse.bacc as bacc
import concourse.tile as tile
from concourse import mybir
from concourse.bass_utils import run_bass_kernel_spmd

F32 = mybir.dt.float32
F32R = mybir.dt.float32r
F16 = mybir.dt.float16
COPY = mybir.ActivationFunctionType.Copy

NFE = 8
STEP = 0.125
N, C, V, L = 64, 64, 500, 12
O = 64
T = NFE + 1          # 9 states
NCORES = 8
NPC = N // NCORES    # 8 batches per core
WT = 4               # node-dim tiles
VTILE = V // WT      # 125
LH = 6               # lags per half-block (cols per Horner matmul = LH*O = 384)
NHB = L // LH        # 2 half-blocks per batch
JT = T * O           # 576 z columns (t*O + o)
JH = JT // 2         # 288, half of the z columns per psum bank
import os
PACK_Z = os.environ.get("PACK_Z", "1") == "1"  # z matmuls as two concurrent K=64 PE row-groups
USE_F16 = os.environ.get("DTYPE", "f16") == "f16"  # fp16 operands: LDWEIGHTS overlaps (fp32r self-load serializes)
DT = F16 if USE_F16 else F32R
NPDT = np.float16 if USE_F16 else np.float32
HP_BUFS = int(os.environ.get("HP_BUFS", "4"))    # Horner psum banks
ZPS_BUFS = int(os.environ.get("ZPS_BUFS", "2"))  # z psum tiles (2 banks each)
Z_AABB = os.environ.get("Z_AABB", "1") == "1"    # same-stationary z matmuls adjacent


def build_nc(repeat=1):
    nc = bacc.Bacc(trn_type="TRN2", target_bir_lowering=False, debug=False)
    x_d = nc.dram_tensor("x", [NPC, C, V, L], DT, kind="ExternalInput")
    mt_d = nc.dram_tensor("mt", [V, V], DT, kind="ExternalInput")
    wr_d = nc.dram_tensor("wr", [C, JT], DT, kind="ExternalInput")
    out_d = nc.dram_tensor("out", [NPC, V, L, O], DT, kind="ExternalOutput")

    with tile.TileContext(nc) as tc, ExitStack() as ctx:
        rep = ctx.enter_context(tc.For_i(0, repeat, 1)) if repeat > 1 else None
        const = ctx.enter_context(tc.tile_pool(name="const", bufs=1))
        xp = ctx.enter_context(tc.tile_pool(name="xp", bufs=2))
        zp = ctx.enter_context(tc.tile_pool(name="zp", bufs=2))
        up = ctx.enter_context(tc.tile_pool(name="up", bufs=12))
        zps = ctx.enter_context(tc.tile_pool(name="zps", bufs=ZPS_BUFS, space="PSUM"))
        hp = ctx.enter_context(tc.tile_pool(name="hp", bufs=HP_BUFS, space="PSUM"))

        # constants: M^T node-mix tiles and the permuted W
        mt_sb = []
        for wt in range(WT):
            t_ = const.tile([VTILE, V], DT, tag=f"mt{wt}", name=f"mt_sb{wt}")
            nc.sync.dma_start(t_[:], mt_d.ap()[wt * VTILE:(wt + 1) * VTILE, :])
            mt_sb.append(t_)
        if PACK_Z:
            wr_sb = const.tile([2 * C, JT], DT, tag="wr", name="wr_sb")
            nc.sync.dma_start(wr_sb[0:C, :], wr_d.ap()[:])
            nc.sync.dma_start(wr_sb[C:2 * C, :], wr_d.ap()[:])
        else:
            wr_sb = const.tile([C, JT], DT, tag="wr", name="wr_sb")
            nc.sync.dma_start(wr_sb[:], wr_d.ap()[:])

        hbs = [(n, hb) for n in range(NPC) for hb in range(NHB)]
        x_tiles = {}
        z_tiles = {}

        def ensure_x(n):
            # With PACK_Z, x is duplicated onto partitions 64..127 so two
            # lags can occupy distinct PE row-groups in concurrent matmuls.
            if n not in x_tiles:
                if PACK_Z:
                    x_tiles[n] = xp.tile([2 * C, V, L], DT, tag="x",
                                         name=f"x_sb_{n}")
                    nc.sync.dma_start(x_tiles[n][0:C], x_d.ap()[n])
                    nc.sync.dma_start(x_tiles[n][C:2 * C], x_d.ap()[n])
                else:
                    x_tiles[n] = xp.tile([C, V, L], DT, tag="x",
                                         name=f"x_sb_{n}")
                    nc.sync.dma_start(x_tiles[n][:], x_d.ap()[n])
            return x_tiles[n]

        def _zcopy(z, wt, li, ps):
            src = ps[:].rearrange("p (b d) -> p b d", b=2)[:, :, 0:JH]
            dst = z[wt][:, li, :].rearrange("p (b d) -> p b d", b=2)
            nc.scalar.activation(dst, src, COPY)

        def make_z_units(j):
            """Closures that emit half-block j's z work:
            z[wt][:, li, t*O+o] = sum_c x[c, w, lo+li] * W[o, t*C+c]."""
            n, hb = hbs[j]
            x_sb = ensure_x(n)
            z = [zp.tile([VTILE, LH, JT], DT, tag=f"z{wt}",
                         name=f"z{wt}_{n}_{hb}") for wt in range(WT)]
            z_tiles[j] = z
            units = []
            if PACK_Z:
                # lag pair (lp, lp+LH//2) on PE row-groups (0,0) / (64,0)
                for lp in range(LH // 2):
                    la, lb = hb * LH + lp, hb * LH + lp + LH // 2
                    for wt in range(WT):
                        def unit(lp=lp, la=la, lb=lb, wt=wt):
                            ws = slice(wt * VTILE, (wt + 1) * VTILE)
                            psa = zps.tile([VTILE, 1024], F32, tag="zps",
                                           name=f"zpa_{n}_{hb}_{lp}_{wt}")
                            psb = zps.tile([VTILE, 1024], F32, tag="zps",
                                           name=f"zpb_{n}_{hb}_{lp}_{wt}")
                            if Z_AABB:
                                # same-stationary matmuls adjacent: the x-slab
                                # weight load hides under the sibling's stream
                                for h in range(2):
                                    cs = slice(h * JH, (h + 1) * JH)
                                    nc.tensor.matmul(
                                        psa[:, h * 512:h * 512 + JH],
                                        x_sb[0:C, ws, la],
                                        wr_sb[0:C, cs], start=True, stop=True,
                                        tile_position=(0, 0))
                                _zcopy(z, wt, lp, psa)
                                for h in range(2):
                                    cs = slice(h * JH, (h + 1) * JH)
                                    nc.tensor.matmul(
                                        psb[:, h * 512:h * 512 + JH],
                                        x_sb[C:2 * C, ws, lb],
                                        wr_sb[C:2 * C, cs], start=True,
                                        stop=True, tile_position=(64, 0))
                                _zcopy(z, wt, lp + LH // 2, psb)
                            else:
                                for h in range(2):
                                    cs = slice(h * JH, (h + 1) * JH)
                                    po = h * 512
                                    nc.tensor.matmul(
                                        psa[:, po:po + JH], x_sb[0:C, ws, la],
                                        wr_sb[0:C, cs], start=True, stop=True,
                                        tile_position=(0, 0))
                                    nc.tensor.matmul(
                                        psb[:, po:po + JH], x_sb[C:2 * C, ws, lb],
                                        wr_sb[C:2 * C, cs], start=True, stop=True,
                                        tile_position=(64, 0))
                                _zcopy(z, wt, lp, psa)
                                _zcopy(z, wt, lp + LH // 2, psb)
                        units.append(unit)
            else:
                for li in range(LH):
                    l = hb * LH + li
                    for wt in range(WT):
                        def unit(li=li, l=l, wt=wt):
                            lhsT = x_sb[:, wt * VTILE:(wt + 1) * VTILE, l]
                            ps = zps.tile([VTILE, 1024], F32, tag="zps",
                                          name=f"zps_{n}_{hb}_{li}_{wt}")
                            nc.tensor.matmul(ps[:, 0:JH], lhsT, wr_sb[:, 0:JH],
                                             start=True, stop=True)
                            nc.tensor.matmul(ps[:, 512:512 + JH], lhsT,
                                             wr_sb[:, JH:JT],
                                             start=True, stop=True)
                            _zcopy(z, wt, li, ps)
                        units.append(unit)
            return units

        # prologue: z for half-block 0 emitted standalone
        for unit in make_z_units(0):
            unit()

        for k, (n, hb) in enumerate(hbs):
            pending = make_z_units(k + 1) if k + 1 < len(hbs) else []
            pace = -(-len(pending) // NFE) if pending else 0  # units per step
            z = z_tiles[k]
            # Horner: u_8 = z_8 (read in place); u_t = z_t + M u_{t+1}
            u = [None] * WT
            for t in range(NFE - 1, -1, -1):
                u_new = [None] * WT
                for vt in range(WT):
                    lhs_col = slice(vt * VTILE, (vt + 1) * VTILE)
                    ps = hp.tile([VTILE, LH, O], F32, tag="hp",
                                 name=f"hps_{n}_{hb}_{t}_{vt}")
                    for wt in range(WT):
                        rhs = (z[wt][:, :, NFE * O:T * O]
                               if t == NFE - 1 else u[wt][:])
                        nc.tensor.matmul(ps[:], mt_sb[wt][:, lhs_col], rhs,
                                         start=(wt == 0), stop=(wt == WT - 1))
                    u_new[vt] = up.tile([VTILE, LH, O], DT, tag="u",
                                        name=f"u_{n}_{hb}_{t}_{vt}")
                    nc.vector.tensor_add(
                        u_new[vt][:], ps[:],
                        z[vt][:, :, t * O:(t + 1) * O])
                    # next half-block's z production fills PE step gaps;
                    # spread at vt granularity to avoid bursty psum demand
                    if pending and vt % 2 == 1 and pace:
                        pending.pop(0)()
                u = u_new
                for _ in range(max(0, pace - 2)):
                    if pending:
                        pending.pop(0)()
            while pending:
                pending.pop(0)()
            del z_tiles[k]

            for vt in range(WT):
                vs = slice(vt * VTILE, (vt + 1) * VTILE)
                nc.sync.dma_start(out_d.ap()[n, vs, hb * LH:(hb + 1) * LH, :],
                                  u[vt][:])
    nc.compile()
    return nc


_NC_CACHE = None


def _get_nc(repeat=1):
    global _NC_CACHE
    if _NC_CACHE is None or _NC_CACHE[0] != repeat:
        _NC_CACHE = (repeat, build_nc(repeat))
    return _NC_CACHE[1]


def kernel(x, adj, W, b, _trace=False, _trace_kwargs=None, _repeat=1):
    x = np.ascontiguousarray(np.asarray(x, dtype=np.float32))
    adj = np.asarray(adj, dtype=np.float32)
    W = np.asarray(W, dtype=np.float32)
    b = np.asarray(b, dtype=np.float32)

    mt = np.ascontiguousarray((np.eye(V, dtype=np.float32) + STEP * adj).T.astype(NPDT))
    wr = np.ascontiguousarray(
        W.reshape(O, T, C).transpose(2, 1, 0).reshape(C, JT).astype(NPDT))
    x = x.astype(NPDT)

    nc = _get_nc(_repeat)
    in_maps = [
        {"x": x[i * NPC:(i + 1) * NPC], "mt": mt, "wr": wr}
        for i in range(NCORES)
    ]
    kw = {}
    if _trace:
        kw["trace"] = True
        kw.update(_trace_kwargs or {})
    res = run_bass_kernel_spmd(nc, in_maps, list(range(NCORES)), **kw)
    out = np.concatenate([res.results[i]["out"] for i in range(NCORES)], axis=0)
    out = out.astype(np.float32).transpose(0, 3, 1, 2)   # [N, O, V, L]
    out = out + b[None, :, None, None]
    if _trace:
        return np.ascontiguousarray(out.astype(np.float32)), res
    return np.ascontiguousarray(out.astype(np.float32))



# revision 6
# speedup vs baseline: 1.1315x; 1.0860x over previous
"""Trainium2 Bass kernel for nn_CGPODE (graph ODE message passing).

Math: reference computes NFE=8 Euler steps of dx/dt = A x over the node
dim (s_t = M^t x with M = I + 0.125*adj applied on the V axis), concats
the 9 states channel-wise, then applies a 1x1 conv (channel GEMM W) + b.

Refactoring used here (per batch n and lag l, x_nl is a [C,V] slab):
    out_nl = sum_t  W_t s_t           (W_t = W[:, t*C:(t+1)*C])
           = sum_t  M^t (W_t x_nl)    (channel mix commutes with node mix)
           = Horner: u_8 = z_8; u_t = z_t + M u_{t+1}; out_nl = u_0
with z_t = W_t-channel-mix of x computed straight from x via the PE
(x slab as the stationary operand), so every tensor stays node-on-partition
and no transposes or state materialization are needed.

Schedule: per batch n, two half-blocks of LH=6 lags. z for the next
half-block (PE matmuls + ACT psum->sbuf copies) overlaps the current
half-block's Horner sweep (PE matmuls + DVE adds); z is double-buffered.

v2: z/u/psum tiles merged so each DVE drain covers a vt-PAIR (one 925ns
op instead of two 557ns ones) and each z unit is drained by ONE ACT copy
(1103ns instead of 2x712ns) — the per-op PSUM-read bubble (120cyc DVE /
172cyc ACT) amortizes over twice the elements.

Sharding: data-parallel over batch N across the 8 cores (adj/W replicated).
"""
import sys
if "/opt/trn_rl_repo" not in sys.path:
    sys.path.append("/opt/trn_rl_repo")  # fallback when axon_site paths absent
from contextlib import ExitStack

import numpy as np

import concourse.bacc as bacc
import concourse.tile as tile
from concourse import mybir
from concourse.bass_utils import run_bass_kernel_spmd

F32 = mybir.dt.float32
F16 = mybir.dt.float16
COPY = mybir.ActivationFunctionType.Copy

NFE = 8
STEP = 0.125
N, C, V, L = 64, 64, 500, 12
O = 64
T = NFE + 1          # 9 states
NCORES = 8
NPC = N // NCORES    # 8 batches per core
WT = 4               # node-dim tiles
VTILE = V // WT      # 125
LH = 6               # lags per half-block (cols per Horner matmul = LH*O = 384)
NHB = L // LH        # 2 half-blocks per batch
JT = T * O           # 576 z columns (t*O + o)
JH = JT // 2         # 288, half of the z columns per psum bank
import os
DT = F16
NPDT = np.float16
ZPAIR = os.environ.get("ZPAIR", "1") == "1"   # one 4-bank zps tile + one ACT copy per unit
UPAIR = os.environ.get("UPAIR", "1") == "1"   # vt-paired 2-bank horner psum + paired DVE drain


def build_nc(repeat=1):
    nc = bacc.Bacc(trn_type="TRN2", target_bir_lowering=False, debug=False)
    x_d = nc.dram_tensor("x", [NPC, C, V, L], DT, kind="ExternalInput")
    mt_d = nc.dram_tensor("mt", [V, V], DT, kind="ExternalInput")
    wr_d = nc.dram_tensor("wr", [C, JT], DT, kind="ExternalInput")
    out_d = nc.dram_tensor("out", [NPC, V, L, O], DT, kind="ExternalOutput")

    with tile.TileContext(nc) as tc, ExitStack() as ctx:
        rep = ctx.enter_context(tc.For_i(0, repeat, 1)) if repeat > 1 else None
        const = ctx.enter_context(tc.tile_pool(name="const", bufs=1))
        xp = ctx.enter_context(tc.tile_pool(name="xp", bufs=2))
        zp = ctx.enter_context(tc.tile_pool(name="zp", bufs=2))
        up = ctx.enter_context(tc.tile_pool(name="up", bufs=3))
        zps = ctx.enter_context(
            tc.tile_pool(name="zps", bufs=1 if ZPAIR else 2, space="PSUM"))
        hp = ctx.enter_context(
            tc.tile_pool(name="hp", bufs=2 if UPAIR else 4, space="PSUM"))

        # constants: M^T node-mix tiles and the permuted W (wr duplicated on
        # partitions 64..127 so two lags can use distinct PE row-groups)
        mt_sb = []
        for wt in range(WT):
            t_ = const.tile([VTILE, V], DT, tag=f"mt{wt}", name=f"mt_sb{wt}")
            nc.sync.dma_start(t_[:], mt_d.ap()[wt * VTILE:(wt + 1) * VTILE, :])
            mt_sb.append(t_)
        wr_sb = const.tile([2 * C, JT], DT, tag="wr", name="wr_sb")
        nc.sync.dma_start(wr_sb[0:C, :], wr_d.ap()[:])
        nc.sync.dma_start(wr_sb[C:2 * C, :], wr_d.ap()[:])

        hbs = [(n, hb) for n in range(NPC) for hb in range(NHB)]
        x_tiles = {}
        z_tiles = {}

        def ensure_x(n):
            # x duplicated onto partitions 64..127 so two lags can occupy
            # distinct PE row-groups in back-to-back matmuls.
            if n not in x_tiles:
                x_tiles[n] = xp.tile([2 * C, V, L], DT, tag="x",
                                     name=f"x_sb_{n}")
                nc.sync.dma_start(x_tiles[n][0:C], x_d.ap()[n])
                nc.sync.dma_start(x_tiles[n][C:2 * C], x_d.ap()[n])
            return x_tiles[n]

        def make_z_units(j):
            """Closures that emit half-block j's z work:
            zbig[:, wt, li, t*O+o] = sum_c x[c, wt*125+w, lo+li] * W[o, t*C+c].
            Unit (lp, wt): lags (la, lb) = (lp, lp+3) of the half-block, four
            288-col matmuls (a-low, a-high, b-low, b-high), one ACT copy."""
            n, hb = hbs[j]
            x_sb = ensure_x(n)
            zbig = zp.tile([VTILE, WT, LH, JT], DT, tag="z",
                           name=f"z_{n}_{hb}")
            z_tiles[j] = zbig
            units = []
            for lp in range(LH // 2):
                la, lb = hb * LH + lp, hb * LH + lp + LH // 2
                for wt in range(WT):
                    def unit(lp=lp, la=la, lb=lb, wt=wt):
                        ws = slice(wt * VTILE, (wt + 1) * VTILE)
                        if ZPAIR:
                            ps = zps.tile([VTILE, 4, 512], F32, tag="zps",
                                          name=f"zps_{n}_{hb}_{lp}_{wt}")
                            for h in range(2):
                                cs = slice(h * JH, (h + 1) * JH)
                                nc.tensor.matmul(
                                    ps[:, h, 0:JH], x_sb[0:C, ws, la],
                                    wr_sb[0:C, cs], start=True, stop=True,
                                    tile_position=(0, 0))
                            for h in range(2):
                                cs = slice(h * JH, (h + 1) * JH)
                                nc.tensor.matmul(
                                    ps[:, 2 + h, 0:JH], x_sb[C:2 * C, ws, lb],
                                    wr_sb[C:2 * C, cs], start=True,
                                    stop=True, tile_position=(64, 0))
                            # one copy: [4, 288] psum chunks -> lags (lp, lp+3)
                            src = ps[:, :, 0:JH].rearrange(
                                "p (g b) d -> p g b d", g=2)
                            dst = zbig[:, wt, lp::LH // 2, :].rearrange(
                                "p g (b d) -> p g b d", b=2)
                            nc.scalar.activation(dst, src, COPY)
                        else:
                            psa = zps.tile([VTILE, 1024], F32, tag="zps",
                                           name=f"zpa_{n}_{hb}_{lp}_{wt}")
                            psb = zps.tile([VTILE, 1024], F32, tag="zps",
                                           name=f"zpb_{n}_{hb}_{lp}_{wt}")
                            for h in range(2):
                                cs = slice(h * JH, (h + 1) * JH)
                                nc.tensor.matmul(
                                    psa[:, h * 512:h * 512 + JH],
                                    x_sb[0:C, ws, la],
                                    wr_sb[0:C, cs], start=True, stop=True,
                                    tile_position=(0, 0))
                            for h in range(2):
                                cs = slice(h * JH, (h + 1) * JH)
                                nc.tensor.matmul(
                                    psb[:, h * 512:h * 512 + JH],
                                    x_sb[C:2 * C, ws, lb],
                                    wr_sb[C:2 * C, cs], start=True,
                                    stop=True, tile_position=(64, 0))
                            for li, pst in ((lp, psa), (lp + LH // 2, psb)):
                                src = pst[:].rearrange(
                                    "p (b d) -> p b d", b=2)[:, :, 0:JH]
                                dst = zbig[:, wt, li, :].rearrange(
                                    "p (b d) -> p b d", b=2)
                                nc.scalar.activation(dst, src, COPY)
                    units.append(unit)
            return units

        # prologue: z for half-block 0 emitted standalone
        for unit in make_z_units(0):
            unit()

        for k, (n, hb) in enumerate(hbs):
            pending = make_z_units(k + 1) if k + 1 < len(hbs) else []
            zbig = z_tiles[k]
            # Horner: u_8 = z_8 (read in place); u_t = z_t + M u_{t+1}
            u = None
            for t in range(NFE - 1, -1, -1):
                u_new = up.tile([VTILE, WT, LH, O], DT, tag="u",
                                name=f"u_{n}_{hb}_{t}")
                for vtp in range(WT // 2):
                    if UPAIR:
                        ps = hp.tile([VTILE, 2, 512], F32, tag="hp",
                                     name=f"hps_{n}_{hb}_{t}_{vtp}")
                        for jj in range(2):
                            vt = 2 * vtp + jj
                            lhs_col = slice(vt * VTILE, (vt + 1) * VTILE)
                            for wt in range(WT):
                                rhs = (zbig[:, wt, :, NFE * O:T * O]
                                       if t == NFE - 1 else u[:, wt])
                                nc.tensor.matmul(
                                    ps[:, jj, 0:LH * O],
                                    mt_sb[wt][:, lhs_col], rhs,
                                    start=(wt == 0), stop=(wt == WT - 1))
                        nc.vector.tensor_add(
                            u_new[:, 2 * vtp:2 * vtp + 2],
                            ps[:, :, 0:LH * O].rearrange(
                                "p g (a b) -> p g a b", a=LH),
                            zbig[:, 2 * vtp:2 * vtp + 2, :,
                                 t * O:(t + 1) * O])
                    else:
                        for jj in range(2):
                            vt = 2 * vtp + jj
                            lhs_col = slice(vt * VTILE, (vt + 1) * VTILE)
                            ps = hp.tile([VTILE, LH, O], F32, tag="hp",
                                         name=f"hps_{n}_{hb}_{t}_{vt}")
                            for wt in range(WT):
                                rhs = (zbig[:, wt, :, NFE * O:T * O]
                                       if t == NFE - 1 else u[:, wt])
                                nc.tensor.matmul(
                                    ps[:], mt_sb[wt][:, lhs_col], rhs,
                                    start=(wt == 0), stop=(wt == WT - 1))
                            nc.vector.tensor_add(
                                u_new[:, vt], ps[:],
                                zbig[:, vt, :, t * O:(t + 1) * O])
                    # next half-block's z production fills PE step gaps
                    if pending:
                        pending.pop(0)()
                u = u_new
            while pending:
                pending.pop(0)()
            del z_tiles[k]

            for vt in range(WT):
                vs = slice(vt * VTILE, (vt + 1) * VTILE)
                nc.sync.dma_start(out_d.ap()[n, vs, hb * LH:(hb + 1) * LH, :],
                                  u[:, vt])
    nc.compile()
    return nc


_NC_CACHE = None


def _get_nc(repeat=1):
    global _NC_CACHE
    if _NC_CACHE is None or _NC_CACHE[0] != repeat:
        _NC_CACHE = (repeat, build_nc(repeat))
    return _NC_CACHE[1]


def kernel(x, adj, W, b, _trace=False, _trace_kwargs=None, _repeat=1):
    x = np.ascontiguousarray(np.asarray(x, dtype=np.float32))
    adj = np.asarray(adj, dtype=np.float32)
    W = np.asarray(W, dtype=np.float32)
    b = np.asarray(b, dtype=np.float32)

    mt = np.ascontiguousarray((np.eye(V, dtype=np.float32) + STEP * adj).T.astype(NPDT))
    wr = np.ascontiguousarray(
        W.reshape(O, T, C).transpose(2, 1, 0).reshape(C, JT).astype(NPDT))
    x = x.astype(NPDT)

    nc = _get_nc(_repeat)
    in_maps = [
        {"x": x[i * NPC:(i + 1) * NPC], "mt": mt, "wr": wr}
        for i in range(NCORES)
    ]
    kw = {}
    if _trace:
        kw["trace"] = True
        kw.update(_trace_kwargs or {})
    res = run_bass_kernel_spmd(nc, in_maps, list(range(NCORES)), **kw)
    out = np.concatenate([res.results[i]["out"] for i in range(NCORES)], axis=0)
    out = out.astype(np.float32).transpose(0, 3, 1, 2)   # [N, O, V, L]
    out = out + b[None, :, None, None]
    if _trace:
        return np.ascontiguousarray(out.astype(np.float32)), res
    return np.ascontiguousarray(out.astype(np.float32))


# revision 7
# speedup vs baseline: 1.1325x; 1.0009x over previous
"""Trainium2 Bass kernel for nn_CGPODE (graph ODE message passing).

Math: reference computes NFE=8 Euler steps of dx/dt = A x over the node
dim (s_t = M^t x with M = I + 0.125*adj applied on the V axis), concats
the 9 states channel-wise, then applies a 1x1 conv (channel GEMM W) + b.

Refactoring used here (per batch n and lag l, x_nl is a [C,V] slab):
    out_nl = sum_t  W_t s_t           (W_t = W[:, t*C:(t+1)*C])
           = sum_t  M^t (W_t x_nl)    (channel mix commutes with node mix)
           = Horner: u_8 = z_8; u_t = z_t + M u_{t+1}; out_nl = u_0
with z_t = W_t-channel-mix of x computed straight from x via the PE
(x slab as the stationary operand), so every tensor stays node-on-partition
and no transposes or state materialization are needed.

Schedule: per batch n, two half-blocks of LH=6 lags. z for the next
half-block (PE matmuls + ACT psum->sbuf copies) overlaps the current
half-block's Horner sweep (PE matmuls + DVE adds); z is double-buffered.

v2: z/u/psum tiles merged so each DVE drain covers a vt-PAIR (one 925ns
op instead of two 557ns ones) and each z unit is drained by ONE ACT copy
(1103ns instead of 2x712ns) — the per-op PSUM-read bubble (120cyc DVE /
172cyc ACT) amortizes over twice the elements.

Sharding: data-parallel over batch N across the 8 cores (adj/W replicated).
"""
import sys
if "/opt/trn_rl_repo" not in sys.path:
    sys.path.append("/opt/trn_rl_repo")  # fallback when axon_site paths absent
from contextlib import ExitStack

import numpy as np

import concourse.bacc as bacc
import concourse.tile as tile
from concourse import mybir
from concourse.bass_utils import run_bass_kernel_spmd

F32 = mybir.dt.float32
F16 = mybir.dt.float16
COPY = mybir.ActivationFunctionType.Copy

NFE = 8
STEP = 0.125
N, C, V, L = 64, 64, 500, 12
O = 64
T = NFE + 1          # 9 states
NCORES = 8
NPC = N // NCORES    # 8 batches per core
WT = 4               # node-dim tiles
VTILE = V // WT      # 125
LH = 6               # lags per half-block (cols per Horner matmul = LH*O = 384)
NHB = L // LH        # 2 half-blocks per batch
JT = T * O           # 576 z columns (t*O + o)
JH = JT // 2         # 288, half of the z columns per psum bank
import os
DT = F16
NPDT = np.float16
ZPAIR = os.environ.get("ZPAIR", "1") == "1"   # one 4-bank zps tile + one ACT copy per unit
UPAIR = os.environ.get("UPAIR", "1") == "1"   # vt-paired 2-bank horner psum + paired DVE drain


def build_nc(repeat=1):
    nc = bacc.Bacc(trn_type="TRN2", target_bir_lowering=False, debug=False)
    x_d = nc.dram_tensor("x", [NPC, C, V, L], DT, kind="ExternalInput")
    mt_d = nc.dram_tensor("mt", [V, V], DT, kind="ExternalInput")
    wr_d = nc.dram_tensor("wr", [C, JT], DT, kind="ExternalInput")
    out_d = nc.dram_tensor("out", [NPC, V, L, O], DT, kind="ExternalOutput")

    with tile.TileContext(nc) as tc, ExitStack() as ctx:
        rep = ctx.enter_context(tc.For_i(0, repeat, 1)) if repeat > 1 else None
        const = ctx.enter_context(tc.tile_pool(name="const", bufs=1))
        xp = ctx.enter_context(tc.tile_pool(name="xp", bufs=2))
        zp = ctx.enter_context(tc.tile_pool(name="zp", bufs=2))
        up = ctx.enter_context(tc.tile_pool(name="up", bufs=3))
        zps = ctx.enter_context(
            tc.tile_pool(name="zps", bufs=1 if ZPAIR else 2, space="PSUM"))
        hp = ctx.enter_context(
            tc.tile_pool(name="hp", bufs=2 if UPAIR else 4, space="PSUM"))

        # constants: M^T node-mix tiles and the permuted W (wr duplicated on
        # partitions 64..127 so two lags can use distinct PE row-groups)
        mt_sb = []
        for wt in range(WT):
            t_ = const.tile([VTILE, V], DT, tag=f"mt{wt}", name=f"mt_sb{wt}")
            nc.sync.dma_start(t_[:], mt_d.ap()[wt * VTILE:(wt + 1) * VTILE, :])
            mt_sb.append(t_)
        wr_sb = const.tile([2 * C, JT], DT, tag="wr", name="wr_sb")
        nc.sync.dma_start(wr_sb[0:C, :], wr_d.ap()[:])
        nc.sync.dma_start(wr_sb[C:2 * C, :], wr_d.ap()[:])

        hbs = [(n, hb) for n in range(NPC) for hb in range(NHB)]
        x_tiles = {}
        z_tiles = {}

        def ensure_x(n):
            # x duplicated onto partitions 64..127 so two lags can occupy
            # distinct PE row-groups in back-to-back matmuls.
            if n not in x_tiles:
                x_tiles[n] = xp.tile([2 * C, V, L], DT, tag="x",
                                     name=f"x_sb_{n}")
                nc.sync.dma_start(x_tiles[n][0:C], x_d.ap()[n])
                nc.sync.dma_start(x_tiles[n][C:2 * C], x_d.ap()[n])
            return x_tiles[n]

        def make_z_units(j):
            """Closures that emit half-block j's z work:
            zbig[:, wt, li, t*O+o] = sum_c x[c, wt*125+w, lo+li] * W[o, t*C+c].
            Unit (lp, wt): lags (la, lb) = (lp, lp+3) of the half-block, four
            288-col matmuls (a-low, a-high, b-low, b-high), one ACT copy."""
            n, hb = hbs[j]
            x_sb = ensure_x(n)
            zbig = zp.tile([VTILE, WT, LH, JT], DT, tag="z",
                           name=f"z_{n}_{hb}")
            z_tiles[j] = zbig
            units = []
            for lp in range(LH // 2):
                la, lb = hb * LH + lp, hb * LH + lp + LH // 2
                for wt in range(WT):
                    def unit(lp=lp, la=la, lb=lb, wt=wt):
                        ws = slice(wt * VTILE, (wt + 1) * VTILE)
                        if ZPAIR:
                            ps = zps.tile([VTILE, 4, 512], F32, tag="zps",
                                          name=f"zps_{n}_{hb}_{lp}_{wt}")
                            for h in range(2):
                                cs = slice(h * JH, (h + 1) * JH)
                                nc.tensor.matmul(
                                    ps[:, h, 0:JH], x_sb[0:C, ws, la],
                                    wr_sb[0:C, cs], start=True, stop=True,
                                    tile_position=(0, 0))
                            for h in range(2):
                                cs = slice(h * JH, (h + 1) * JH)
                                nc.tensor.matmul(
                                    ps[:, 2 + h, 0:JH], x_sb[C:2 * C, ws, lb],
                                    wr_sb[C:2 * C, cs], start=True,
                                    stop=True, tile_position=(64, 0))
                            # one copy: [4, 288] psum chunks -> lags (lp, lp+3)
                            src = ps[:, :, 0:JH].rearrange(
                                "p (g b) d -> p g b d", g=2)
                            dst = zbig[:, wt, lp::LH // 2, :].rearrange(
                                "p g (b d) -> p g b d", b=2)
                            nc.scalar.activation(dst, src, COPY)
                        else:
                            psa = zps.tile([VTILE, 1024], F32, tag="zps",
                                           name=f"zpa_{n}_{hb}_{lp}_{wt}")
                            psb = zps.tile([VTILE, 1024], F32, tag="zps",
                                           name=f"zpb_{n}_{hb}_{lp}_{wt}")
                            for h in range(2):
                                cs = slice(h * JH, (h + 1) * JH)
                                nc.tensor.matmul(
                                    psa[:, h * 512:h * 512 + JH],
                                    x_sb[0:C, ws, la],
                                    wr_sb[0:C, cs], start=True, stop=True,
                                    tile_position=(0, 0))
                            for h in range(2):
                                cs = slice(h * JH, (h + 1) * JH)
                                nc.tensor.matmul(
                                    psb[:, h * 512:h * 512 + JH],
                                    x_sb[C:2 * C, ws, lb],
                                    wr_sb[C:2 * C, cs], start=True,
                                    stop=True, tile_position=(64, 0))
                            for li, pst in ((lp, psa), (lp + LH // 2, psb)):
                                src = pst[:].rearrange(
                                    "p (b d) -> p b d", b=2)[:, :, 0:JH]
                                dst = zbig[:, wt, li, :].rearrange(
                                    "p (b d) -> p b d", b=2)
                                nc.scalar.activation(dst, src, COPY)
                    units.append(unit)
            return units

        # prologue: z for half-block 0 emitted standalone
        for unit in make_z_units(0):
            unit()

        for k, (n, hb) in enumerate(hbs):
            pending = make_z_units(k + 1) if k + 1 < len(hbs) else []
            zbig = z_tiles[k]
            # Horner: u_8 = z_8 (read in place); u_t = z_t + M u_{t+1}
            u = None
            for t in range(NFE - 1, -1, -1):
                u_new = up.tile([VTILE, WT, LH, O], DT, tag="u",
                                name=f"u_{n}_{hb}_{t}")
                for vtp in range(WT // 2):
                    if UPAIR:
                        ps = hp.tile([VTILE, 2, 512], F32, tag="hp",
                                     name=f"hps_{n}_{hb}_{t}_{vtp}")
                        for jj in range(2):
                            vt = 2 * vtp + jj
                            lhs_col = slice(vt * VTILE, (vt + 1) * VTILE)
                            for wt in range(WT):
                                rhs = (zbig[:, wt, :, NFE * O:T * O]
                                       if t == NFE - 1 else u[:, wt])
                                nc.tensor.matmul(
                                    ps[:, jj, 0:LH * O],
                                    mt_sb[wt][:, lhs_col], rhs,
                                    start=(wt == 0), stop=(wt == WT - 1))
                        nc.vector.tensor_add(
                            u_new[:, 2 * vtp:2 * vtp + 2],
                            ps[:, :, 0:LH * O].rearrange(
                                "p g (a b) -> p g a b", a=LH),
                            zbig[:, 2 * vtp:2 * vtp + 2, :,
                                 t * O:(t + 1) * O])
                    else:
                        for jj in range(2):
                            vt = 2 * vtp + jj
                            lhs_col = slice(vt * VTILE, (vt + 1) * VTILE)
                            ps = hp.tile([VTILE, LH, O], F32, tag="hp",
                                         name=f"hps_{n}_{hb}_{t}_{vt}")
                            for wt in range(WT):
                                rhs = (zbig[:, wt, :, NFE * O:T * O]
                                       if t == NFE - 1 else u[:, wt])
                                nc.tensor.matmul(
                                    ps[:], mt_sb[wt][:, lhs_col], rhs,
                                    start=(wt == 0), stop=(wt == WT - 1))
                            nc.vector.tensor_add(
                                u_new[:, vt], ps[:],
                                zbig[:, vt, :, t * O:(t + 1) * O])
                    # next half-block's z production fills PE step gaps;
                    # one batch of 2 units per t-step halves the number of
                    # z<->horner PE transitions (each costs an unhidden
                    # weight load: z LDs conflict with in-flight full-row
                    # horner matmuls, so they cannot pull ahead)
                    if vtp == 1:
                        for _ in range(2):
                            if pending:
                                pending.pop(0)()
                u = u_new
            while pending:
                pending.pop(0)()
            del z_tiles[k]

            for vt in range(WT):
                vs = slice(vt * VTILE, (vt + 1) * VTILE)
                nc.sync.dma_start(out_d.ap()[n, vs, hb * LH:(hb + 1) * LH, :],
                                  u[:, vt])
    nc.compile()
    return nc


_NC_CACHE = None


def _get_nc(repeat=1):
    global _NC_CACHE
    if _NC_CACHE is None or _NC_CACHE[0] != repeat:
        _NC_CACHE = (repeat, build_nc(repeat))
    return _NC_CACHE[1]


def kernel(x, adj, W, b, _trace=False, _trace_kwargs=None, _repeat=1):
    x = np.ascontiguousarray(np.asarray(x, dtype=np.float32))
    adj = np.asarray(adj, dtype=np.float32)
    W = np.asarray(W, dtype=np.float32)
    b = np.asarray(b, dtype=np.float32)

    mt = np.ascontiguousarray((np.eye(V, dtype=np.float32) + STEP * adj).T.astype(NPDT))
    wr = np.ascontiguousarray(
        W.reshape(O, T, C).transpose(2, 1, 0).reshape(C, JT).astype(NPDT))
    x = x.astype(NPDT)

    nc = _get_nc(_repeat)
    in_maps = [
        {"x": x[i * NPC:(i + 1) * NPC], "mt": mt, "wr": wr}
        for i in range(NCORES)
    ]
    kw = {}
    if _trace:
        kw["trace"] = True
        kw.update(_trace_kwargs or {})
    res = run_bass_kernel_spmd(nc, in_maps, list(range(NCORES)), **kw)
    out = np.concatenate([res.results[i]["out"] for i in range(NCORES)], axis=0)
    out = out.astype(np.float32).transpose(0, 3, 1, 2)   # [N, O, V, L]
    out = out + b[None, :, None, None]
    if _trace:
        return np.ascontiguousarray(out.astype(np.float32)), res
    return np.ascontiguousarray(out.astype(np.float32))
